# revision 27
# baseline (speedup 1.0000x reference)
"""Correlation (9x9 displacement) kernel for Trainium2.

out[b,c,i,j,y,x] = leaky_relu(ref[b,c,y,x] * tgt[b,c, y+j-4, x+i-4], 0.1)
with zero padding outside the target image bounds.

Sharding: the 256 (b,c) images are split 32-per-core across 8 NeuronCores
(pure data parallel, no collectives).

Per-core layout: partition p = yb*32 + n  (yb = row-block 0..3, n = image
0..31).  Each partition stores a halo tile of the target: 24 rows x 136 cols
(its 16-row block plus +-4 halo rows, W plus +-4 pad cols, zeros outside the
image).  Every displacement (i,j) then becomes the plain in-bounds slice
tgt[:, j:j+16, i:i+128], and out-of-bounds zeros compute themselves via
leaky(ref*0) == 0.  The halo construction happens on the host so each core
issues exactly two flat input DMAs.
"""

import numpy as np

import concourse.bacc as bacc
import concourse.bass as bass
import concourse.mybir as mybir
from concourse import bass_utils
from concourse.tile import TileContext

B, C, H, W = 4, 64, 64, 128
MD = 4
D = 2 * MD + 1  # 9
N_CORES = 8
IMGS = B * C  # 256
IPC = IMGS // N_CORES  # 32 images per core
YB = 4  # row blocks per image
BH = H // YB  # 16 rows per block
HALO_H = BH + 2 * MD  # 24
HALO_W = W + 2 * MD  # 136
JG = 3  # j-group size per ACT op / output DMA

F32 = mybir.dt.float32


def _build(
    jg: int = JG,
    mul_bufs: int = 3,
    out_bufs: int = 3,
    big_out: bool = False,
    skip_zeros: bool = False,
    gp_js: int = 0,
    act_prefetch: bool = False,
    split_in: bool = False,
) -> bass.Bass:
    nc = bacc.Bacc(trn_type="TRN2")
    ref_d = nc.dram_tensor("ref", [128, BH, W], F32, kind="ExternalInput")
    tgt_d = nc.dram_tensor("tgt", [128, HALO_H, HALO_W], F32, kind="ExternalInput")
    # Partition-major output: [p = yb*32+n, i, j, y_lo, x].  Keeps the store
    # DMA at 3 AP dims with 72KB-contiguous per-partition runs; the host
    # untangles (yb, n) during unsharding.
    out_d = nc.dram_tensor("out", [128, D, D, BH, W], F32, kind="ExternalOutput")

    with TileContext(nc) as tc:
        with (
            tc.tile_pool(name="const", bufs=1) as cpool,
            tc.tile_pool(name="mul", bufs=mul_bufs) as mpool,
            tc.tile_pool(name="outp", bufs=out_bufs) as opool,
        ):
            tgt_t = cpool.tile([128, HALO_H, HALO_W], F32)
            ref_t = cpool.tile([128, BH, W], F32)
            if act_prefetch:
                # Touch the Prelu table set before any data arrives so the
                # ~2.7us ACT_TABLE_LOAD overlaps the input DMAs.
                warm = cpool.tile([128, 1], F32)
                nc.vector.memset(warm[:], 0.0)
                nc.scalar.activation(
                    out=warm[:],
                    in_=warm[:],
                    func=mybir.ActivationFunctionType.Prelu,
                    alpha=0.1,
                )
            nc.sync.dma_start(out=ref_t[:], in_=ref_d[:])
            if split_in:
                nc.sync.dma_start(out=tgt_t[:, :BH], in_=tgt_d[:, :BH])
                nc.sync.dma_start(out=tgt_t[:, BH:], in_=tgt_d[:, BH:])
            else:
                nc.sync.dma_start(out=tgt_t[:], in_=tgt_d[:])
            for i in range(D):
                ot_big = None
                if big_out:
                    ot_big = opool.tile([128, D, BH, W], F32, name="otb", tag="otb")
                for jg_i in range(D // jg):
                    mt = mpool.tile([128, jg, BH, W], F32)
                    for jj in range(jg):
                        j = jg_i * jg + jj
                        # Offload the first gp_js j's of each i to GPSIMD to
                        # offload the vector engine.
                        eng = nc.gpsimd if j < gp_js else nc.vector
                        eng.tensor_tensor(
                            out=mt[:, jj],
                            in0=ref_t[:],
                            in1=tgt_t[:, j : j + BH, i : i + W],
                            op=mybir.AluOpType.mult,
                        )
                    if big_out:
                        ot = ot_big[:, jg_i * jg : (jg_i + 1) * jg]
                    else:
                        ot_t = opool.tile([128, jg, BH, W], F32, name="ot", tag="ot")
                        ot = ot_t[:]
                    nc.scalar.activation(
                        out=ot,
                        in_=mt[:],
                        func=mybir.ActivationFunctionType.Prelu,
                        alpha=0.1,
                    )
                    if not big_out:
                        j0 = jg_i * jg
                        dj = j0 - MD
                        if skip_zeros and jg == 1 and dj != 0:
                            # Rows with y+dj out of [0,H) are structural zeros;
                            # the output buffer is pre-zeroed, so skip writing
                            # them.  They live in one partition block (yb=0
                            # for dj<0, yb=3 for dj>0), so the store splits
                            # into two contiguous DMAs.
                            if dj < 0:
                                nc.sync.dma_start(
                                    out=out_d[IPC:, i, j0], in_=ot[IPC:, 0]
                                )
                                nc.sync.dma_start(
                                    out=out_d[:IPC, i, j0, -dj:],
                                    in_=ot[:IPC, 0, -dj:],
                                )
                            else:
                                nc.sync.dma_start(
                                    out=out_d[: 3 * IPC, i, j0], in_=ot[: 3 * IPC, 0]
                                )
                                nc.sync.dma_start(
                                    out=out_d[3 * IPC :, i, j0, : BH - dj],
                                    in_=ot[3 * IPC :, 0, : BH - dj],
                                )
                        else:
                            nc.sync.dma_start(
                                out=out_d[:, i, j0 : j0 + jg],
                                in_=ot,
                            )
                if big_out:
                    nc.sync.dma_start(out=out_d[:, i], in_=ot_big[:])
    nc.finalize()
    return nc


_cached_nc = None
_last_results = None


def _prep_inputs(ref: np.ndarray, tgt: np.ndarray):
    """ref/tgt: (256, 64, 128) f32 -> per-core blocked/halo'd arrays.

    Returns ref_blocked (8, 128, 16, 128) and tgt_halo (8, 128, 24, 136),
    partition p = yb*32 + n.
    """
    # ref: (8 cores, 32 n, 4 yb, 16 y, 128 x) -> (8, yb, n, y, x)
    ref_b = ref.reshape(N_CORES, IPC, YB, BH, W).transpose(0, 2, 1, 3, 4)
    ref_b = np.ascontiguousarray(ref_b).reshape(N_CORES, 128, BH, W)

    tp = np.zeros((IMGS, H + 2 * MD, HALO_W), dtype=np.float32)
    tp[:, MD : MD + H, MD : MD + W] = tgt
    # overlapping 24-row windows starting at yb*16
    idx = (BH * np.arange(YB))[:, None] + np.arange(HALO_H)[None, :]
    halo = tp[:, idx, :]  # (256, 4, 24, 136)
    halo = halo.reshape(N_CORES, IPC, YB, HALO_H, HALO_W).transpose(0, 2, 1, 3, 4)
    halo = np.ascontiguousarray(halo).reshape(N_CORES, 128, HALO_H, HALO_W)
    return ref_b, halo


def kernel(refimg_fea: np.ndarray, targetimg_fea: np.ndarray) -> np.ndarray:
    global _cached_nc, _last_results
    ref = np.asarray(refimg_fea, dtype=np.float32).reshape(IMGS, H, W)
    tgt = np.asarray(targetimg_fea, dtype=np.float32).reshape(IMGS, H, W)
    ref_b, tgt_h = _prep_inputs(ref, tgt)
    if _cached_nc is None:
        _cached_nc = _build(
            jg=1, mul_bufs=6, out_bufs=6, skip_zeros=True, split_in=True
        )
    nc = _cached_nc
    in_maps = [{"ref": ref_b[k], "tgt": tgt_h[k]} for k in range(N_CORES)]
    res = bass_utils.run_bass_kernel_spmd(nc, in_maps, core_ids=list(range(N_CORES)))
    _last_results = res
    # Per-core output is [yb*32+n, i, j, y_lo, x]; reassemble to
    # [n, i, j, (yb y_lo), x] per core, then stack cores along n.
    parts = []
    for r in res.results:
        o = r["out"].reshape(YB, IPC, D, D, BH, W)
        parts.append(o.transpose(1, 2, 3, 0, 4, 5).reshape(IPC, D, D, H, W))
    out = np.concatenate(parts, axis=0)
    return out.reshape(B, C, D, D, H, W)
